# revision 24
# baseline (speedup 1.0000x reference)
"""Two-layer GCN (PyG GCNConv semantics) on 8 Trainium2 NeuronCores.

Sharding: nodes are partitioned into 8 contiguous blocks of B=12544 (padded
N=100352); core c owns node block c and all edges whose dst is in the block.
Both layers aggregate in 128-feature space (layer 2 uses linearity:
sum_e norm*(z1@W2)[src] == (sum_e norm*z1[src])@W2), so every gather table,
AllGather and selection matmul is bf16 with 256B rows:

    hhat1 = d_inv[:,None] * (x @ W1)
    z1hat = d_inv * relu(d_inv * (sum hhat1[src] + hhat1[v]) + b1)
    aggh2 = sum z1hat[src] + z1hat[v]
    z     = relu(d_inv * (aggh2 @ W2) + b2)

Edges are packed tight per (dst-tile-pair, src-sub) group and padded with
idx=-1: the SWDGE gather ucode trims trailing negative indices at runtime,
so per-core padding costs no descriptors and no DMA. Selection one-hots are
built with ONE DVE is_equal per dst tile: the dstl table stores
dst-slot-in-pair minus tile_rel*128 per (tile, sub, chunk) column, so each
tile compares against the same 0..127 iota.
"""

import os
import sys

sys.path.insert(0, "/opt/trn_rl_repo")

import ml_dtypes
import numpy as np

import concourse.bacc as bacc
import concourse.tile as tile
from concourse import bass_utils, mybir
from concourse.library_config import mlp

# ---------------------------------------------------------------------------
# Tile assigns Pool-engine (SWDGE) DMAs to the 8 DMASW semaphore lanes
# round-robin, ignoring queue_num. DMAs on different SWDGE queues complete
# out of order relative to each other, so a lane shared by two queues makes
# the cumulative wait thresholds unsound (CoreSim flags exactly this).
# Patch the lane assignment so each queue owns two dedicated lanes.
import concourse.tile_sem_assignment as _tsa
from concourse.tile_scheduler import DMAInst as _DMAInst

if not getattr(_tsa.TileClockTick, "_qaware_patched", False):
    _orig_assign_tick = _tsa.TileClockTick._assign_tick

    def _assign_tick_qaware(self, inst):
        from concourse import bass_isa as _bisa, mybir as _mb
        if (
            isinstance(inst, _DMAInst)
            and not isinstance(inst, _bisa.UserSyncedRemoteDMADescs)
            and inst.engine == _mb.EngineType.Pool
            and self.swdge_sem_count == 8
        ):
            q = int(getattr(inst, "queue_num", 0) or 0) % 4
            cnts = getattr(self, "_q_lane_cnt", None)
            if cnts is None:
                cnts = self._q_lane_cnt = [0, 0, 0, 0]
            self.next_sw_dma_idx = q * 2 + (cnts[q] % 2)
            cnts[q] += 1
        return _orig_assign_tick(self, inst)

    _tsa.TileClockTick._assign_tick = _assign_tick_qaware
    _tsa.TileClockTick._qaware_patched = True
# ---------------------------------------------------------------------------

# ---------------------------------------------------------------- constants
N = 100000
CIN, CHID, COUT = 128, 128, 64
NCORES = 8
B = 12544                 # nodes per core (98 tiles of 128)
NP = NCORES * B           # padded node count = 100352
TILES = B // 128          # 98 dst tiles per core
# chunk sizes rebalanced: small first chunk (earlier first gather) and
# small last chunk (smaller AG tail bubble); middles take the slack.
TPC = [13, 31, 31, 23]    # tiles per AllGather chunk (sum = 98)
TSTART = [0, 13, 44, 75]  # chunk start tile
NSUB = 4                  # gather sub-tables == AG chunks
PAIR = 4                  # dst tiles merged per gather instruction
NPAIR = -(-TILES // PAIR)
DT = mybir.dt.bfloat16

_F32 = mybir.dt.float32
_I16 = mybir.dt.int16
_CUM_TPC = np.cumsum(TPC)
PAD_DSTL = 999.0


def _chunk_of_tile(t):
    return int(np.searchsorted(_CUM_TPC, t, side="right"))


# ---------------------------------------------------------------- host prep
def _prep(edge_index):
    """Pack edges tight per (core, pair, sub); build per-core idx/dstl and a
    static (shared) gather + matmul schedule."""
    src = edge_index[0].astype(np.int64)
    dst = edge_index[1].astype(np.int64)
    E = src.shape[0]

    deg = np.bincount(dst, minlength=NP).astype(np.float32) + 1.0

    core = dst // B
    dstl = dst - core * B
    t = dstl >> 7
    slot = dstl & 127
    p = t // PAIR
    t_rel = t - p * PAIR
    dip = t_rel * 128 + slot                    # dst-slot-in-pair [0,512)

    csrc = src // B
    lsrc = src - csrc * B
    tsrc = lsrc >> 7
    psrc = lsrc & 127
    s = np.searchsorted(_CUM_TPC, tsrc, side="right")
    tpc_arr = np.asarray(TPC)
    tstart_arr = np.asarray(TSTART)
    row = csrc * (tpc_arr[s] * 128) + (tsrc - tstart_arr[s]) * 128 + psrc

    # group (core, pair, sub); edges sorted by group then tile (stable)
    gid = (core * NPAIR + p) * NSUB + s
    order = np.lexsort((t, gid))
    gid_s = gid[order]
    t_s = t[order]
    row_s = row[order]
    dip_s = dip[order]
    s_s = s[order]

    ngroups = NCORES * NPAIR * NSUB
    cnt = np.bincount(gid_s, minlength=ngroups).reshape(NCORES, NPAIR, NSUB)
    K = np.maximum(1, -(-cnt.max(axis=0) // 128))            # [NPAIR, NSUB]
    kmax_g = [int(K[:, ss].max()) for ss in range(NSUB)]

    slot_base = np.zeros((NPAIR, NSUB), np.int64)
    acc = 0
    for pp in range(NPAIR):
        for ss in range(NSUB):
            slot_base[pp, ss] = acc
            acc += K[pp, ss] * 128
    total_slots = acc

    grp_start = np.zeros(ngroups + 1, np.int64)
    np.cumsum(cnt.reshape(-1), out=grp_start[1:])
    pos = np.arange(E) - grp_start[gid_s]
    # flat slot per (sorted) edge
    pp_e = (gid_s // NSUB) % NPAIR
    ss_e = gid_s % NSUB
    flat = slot_base[pp_e, ss_e] + pos
    core_e = gid_s // (NPAIR * NSUB)

    # per-core per (t, s): start/end position inside the (p, s) group
    cnt_ts = np.zeros((NCORES, TILES, NSUB), np.int64)
    np.add.at(cnt_ts, (core_e, t_s, s_s), 1)
    start_ts = np.zeros_like(cnt_ts)
    for pp in range(NPAIR):
        tt0 = pp * PAIR
        tt1 = min(tt0 + PAIR, TILES)
        run = np.zeros((NCORES, NSUB), np.int64)
        for tt in range(tt0, tt1):
            start_ts[:, tt, :] = run
            run += cnt_ts[:, tt, :]
    end_ts = start_ts + cnt_ts

    # static union chunk range per (t, s) over cores with edges
    any_c = cnt_ts > 0                                        # [C, T, S]
    lo_all = start_ts // 128
    hi_all = -(-end_ts // 128)
    lo_ts = np.where(any_c, lo_all, 10 ** 9).min(axis=0)      # [T, S]
    hi_ts = np.where(any_c, hi_all, -1).max(axis=0)
    empty = ~any_c.any(axis=0)
    lo_ts[empty] = 0
    hi_ts[empty] = 0
    rng_ts = (hi_ts - lo_ts).astype(np.int64)                 # [T, S]
    n_mm_t = rng_ts.sum(axis=1)                               # [T]
    kmax_sel = int(n_mm_t.max())
    selbase_t = np.zeros(TILES + 1, np.int64)
    np.cumsum(n_mm_t, out=selbase_t[1:])
    n_dstl = int(selbase_t[-1])

    # dstl column -> (global chunk index, value offset) map (static)
    chunk_map = np.zeros(n_dstl, np.int64)
    off_map = np.zeros(n_dstl, np.float32)
    for tt in range(TILES):
        pp = tt // PAIR
        c0 = selbase_t[tt]
        for ss in range(NSUB):
            r = rng_ts[tt, ss]
            if r == 0:
                continue
            gchunk0 = slot_base[pp, ss] // 128 + lo_ts[tt, ss]
            chunk_map[c0:c0 + r] = np.arange(gchunk0, gchunk0 + r)
            off_map[c0:c0 + r] = (tt % PAIR) * 128
            c0 += r

    # per-core arrays
    idx_arrs, dstl_arrs, deg_cols = [], [], []
    w16 = total_slots // 16
    n_chunks_tot = total_slots // 128
    for c in range(NCORES):
        mask = core_e == c
        fl = flat[mask]
        idx_flat = np.full(total_slots, 0, np.int16)
        dip_full = np.full(total_slots, PAD_DSTL, np.float32)
        idx_flat[fl] = row_s[mask].astype(np.int16)
        dip_full[fl] = dip_s[mask].astype(np.float32)

        iw = idx_flat.reshape(w16, 16).T                      # [16, w16]
        idx_arrs.append(np.tile(iw, (8, 1)).astype(np.int16))

        chunks = dip_full.reshape(n_chunks_tot, 128)          # [chunks, 128]
        dcols = chunks[chunk_map].T - off_map[None, :]        # [128, n_dstl]
        dstl_arrs.append(np.ascontiguousarray(dcols.astype(np.float32)))

        deg_cols.append(np.ascontiguousarray(
            deg[c * B:(c + 1) * B].reshape(TILES, 128).T))

    gather_sched = [[(int(slot_base[pp, ss]), int(K[pp, ss]))
                     for ss in range(NSUB)] for pp in range(NPAIR)]
    tile_sched = [
        dict(selbase=int(selbase_t[tt]), n_mm=int(n_mm_t[tt]),
             rng=[(int(lo_ts[tt, ss]), int(rng_ts[tt, ss]))
                  for ss in range(NSUB)])
        for tt in range(TILES)
    ]
    return (idx_arrs, dstl_arrs, deg_cols, gather_sched, tile_sched,
            n_dstl, total_slots, kmax_g, kmax_sel)


# ---------------------------------------------------------------- device IR
def _build(gather_sched, tile_sched, n_dstl, total_slots, kmax_g, kmax_sel,
           skip_bias):
    nc = bacc.Bacc(
        "TRN2",
        target_bir_lowering=False,
        debug=False,
        num_devices=NCORES,
        num_swdge_queues=4,
    )

    w16 = total_slots // 16
    xt_t = nc.dram_tensor("xt", [128, B], DT, kind="ExternalInput")
    idx_t = nc.dram_tensor("idx", [128, w16], _I16, kind="ExternalInput")
    dstl_t = nc.dram_tensor("dstl", [128, n_dstl], _F32, kind="ExternalInput")
    deg_t = nc.dram_tensor("deg", [128, TILES], _F32, kind="ExternalInput")
    w1_t = nc.dram_tensor("w1", [CIN, CHID], _F32, kind="ExternalInput")
    w2_t = nc.dram_tensor("w2", [CHID, COUT], _F32, kind="ExternalInput")
    b1_t = nc.dram_tensor("b1", [128, CHID], _F32, kind="ExternalInput")
    b2_t = nc.dram_tensor("b2", [128, COUT], _F32, kind="ExternalInput")
    ident_t = nc.dram_tensor("ident", [128, 128], _F32, kind="ExternalInput")
    iotar_t = nc.dram_tensor("iotar", [128, kmax_sel * 128], _F32,
                             kind="ExternalInput")
    z_t = nc.dram_tensor("z", [B, COUT], _F32, kind="ExternalOutput")

    rg = [list(range(NCORES))]

    with tile.TileContext(nc) as tc:
        with (
            tc.tile_pool(name="const", bufs=1) as cpool,
            tc.tile_pool(name="sel", bufs=3) as spool,
            tc.tile_pool(name="gath", bufs=3) as gpool,
            tc.tile_pool(name="zeps", bufs=4) as zpool,
            tc.tile_pool(name="hh2", bufs=3) as hpool,
            tc.tile_pool(name="ps", bufs=2, space="PSUM") as ppool,
            tc.tile_pool(name="dram", bufs=1, space="DRAM") as dpool,
        ):
            nc.gpsimd.load_library(mlp)

            # ---- constants / inputs staged once
            idx_sb = cpool.tile([128, w16], _I16)
            nc.sync.dma_start(idx_sb[:], idx_t[:])
            dstl_f = cpool.tile([128, n_dstl], _F32)
            nc.sync.dma_start(dstl_f[:], dstl_t[:])
            dstl_sb = cpool.tile([128, n_dstl], DT)
            nc.vector.tensor_copy(dstl_sb[:], dstl_f[:])
            deg_sb = cpool.tile([128, TILES], _F32)
            nc.sync.dma_start(deg_sb[:], deg_t[:])
            w1_sb = cpool.tile([CIN, CHID], _F32)
            nc.sync.dma_start(w1_sb[:], w1_t[:])
            w2_sb = cpool.tile([CHID, COUT], _F32)
            nc.sync.dma_start(w2_sb[:], w2_t[:])
            w1b = cpool.tile([CIN, CHID], DT)
            nc.vector.tensor_copy(w1b[:], w1_sb[:])
            w2b = cpool.tile([CHID, COUT], DT)
            nc.vector.tensor_copy(w2b[:], w2_sb[:])
            b1_sb = cpool.tile([128, CHID], _F32)
            nc.sync.dma_start(b1_sb[:], b1_t[:])
            b2_sb = cpool.tile([128, COUT], _F32)
            nc.sync.dma_start(b2_sb[:], b2_t[:])
            identF = cpool.tile([128, 128], _F32)
            nc.sync.dma_start(identF[:], ident_t[:])
            ident1 = cpool.tile([128, 128], DT)
            nc.vector.tensor_copy(ident1[:], identF[:])
            iotaF = cpool.tile([128, kmax_sel * 128], _F32)
            nc.sync.dma_start(iotaF[:], iotar_t[:])
            iota1 = cpool.tile([128, kmax_sel * 128], DT)
            nc.vector.tensor_copy(iota1[:], iotaF[:])

            # d_inv = sqrt(1/deg)
            recip = cpool.tile([128, TILES], _F32)
            nc.vector.reciprocal(recip[:], deg_sb[:])
            dinv = cpool.tile([128, TILES], _F32)
            nc.scalar.activation(dinv[:], recip[:],
                                 mybir.ActivationFunctionType.Sqrt)

            # SBUF-resident x^T, hhat1, z1hat (per-tile slices)
            xfull = cpool.tile([128, B], DT)
            for ss in range(NSUB):
                a = TSTART[ss] * 128
                bb = a + TPC[ss] * 128
                nc.sync.dma_start(xfull[:, a:bb], xt_t[:, a:bb])
            hh_all = cpool.tile([128, B], DT)
            zh_all = cpool.tile([128, B], DT)

            # warm the gather buffers so runtime-trimmed (unwritten) slots
            # hold finite values (sel==0 against NaN garbage would poison
            # PSUM)
            gwarm = []
            for ss in range(NSUB):
                for _ in range(3):
                    g_sb = gpool.tile([128, kmax_g[ss], CHID], DT,
                                      tag=f"g{ss}")
                    nc.vector.memset(g_sb[:], 0)
                    gwarm.append(g_sb)

            # ---- DRAM buffers: AG inputs (local tables) and gather tables
            agin1 = [dpool.tile([TPC[s] * 128, CHID], DT, name=f"agin1_{s}")
                     for s in range(NSUB)]
            h1tab = [dpool.tile([NCORES * TPC[s] * 128, CHID], DT,
                                name=f"h1tab_{s}") for s in range(NSUB)]
            agin2 = [dpool.tile([TPC[s] * 128, CHID], DT, name=f"agin2_{s}")
                     for s in range(NSUB)]
            h2tab = [dpool.tile([NCORES * TPC[s] * 128, CHID], DT,
                                name=f"h2tab_{s}") for s in range(NSUB)]

            # ---------------- phase 1: hhat1 = d_inv * (x @ W1), local rows
            for t in range(TILES):
                ps = ppool.tile([128, CHID], _F32, tag="pagg", bufs=3)
                nc.tensor.matmul(out=ps[:],
                                 lhsT=xfull[:, t * 128:(t + 1) * 128],
                                 rhs=w1b[:], start=True, stop=True)
                nc.vector.tensor_scalar(
                    out=hh_all[:, t * 128:(t + 1) * 128], in0=ps[:],
                    scalar1=dinv[:, t:t + 1],
                    scalar2=None, op0=mybir.AluOpType.mult)
                s = _chunk_of_tile(t)
                r0 = (t - TSTART[s]) * 128
                nc.sync.dma_start(agin1[s][r0:r0 + 128, :],
                                  hh_all[:, t * 128:(t + 1) * 128])

            for s in range(NSUB):
                nc.gpsimd.collective_compute(
                    "AllGather", mybir.AluOpType.bypass, replica_groups=rg,
                    ins=[agin1[s].opt()], outs=[h1tab[s].opt()])

            # ---------------- aggregation layer (per pair of dst tiles)
            def emit_gather(pp, ss, tabs):
                sbase, kps = gather_sched[pp][ss]
                g_sb = gpool.tile([128, kmax_g[ss], CHID], DT, tag=f"g{ss}")
                o16 = sbase // 16
                nc.gpsimd.dma_gather(
                    g_sb[:, :kps, :], tabs[ss][:],
                    idx_sb[:, o16:o16 + kps * 8],
                    kps * 128, kps * 128, CHID,
                    single_packet=False, queue_num=(pp + ss) % 4)
                return g_sb

            def agg_pair(pp, gbufs, loc_all, epilogue):
                for t in range(pp * PAIR, min((pp + 1) * PAIR, TILES)):
                    ts = tile_sched[t]
                    n_mm = ts["n_mm"]
                    ps = ppool.tile([128, CHID], _F32, tag="pagg", bufs=3)
                    # self-loop (outer d_inv scale supplies the square)
                    nc.tensor.matmul(out=ps[:], lhsT=ident1[:],
                                     rhs=loc_all[:, t * 128:(t + 1) * 128],
                                     start=True, stop=(n_mm == 0))
                    if n_mm:
                        sb = ts["selbase"]
                        sel = spool.tile([128, kmax_sel, 128], DT, tag="sel")
                        nc.vector.tensor_tensor(
                            out=sel[:, :n_mm, :],
                            in0=iota1[:, :n_mm * 128].rearrange(
                                "p (k c) -> p k c", c=128),
                            in1=dstl_sb[:, sb:sb + n_mm].to_broadcast(
                                [128, n_mm, 128]),
                            op=mybir.AluOpType.is_equal)
                        q = 0
                        for ss in range(NSUB):
                            lo, r = ts["rng"][ss]
                            for jj in range(r):
                                q += 1
                                nc.tensor.matmul(
                                    out=ps[:], lhsT=sel[:, q - 1, :],
                                    rhs=gbufs[ss][:, lo + jj, :],
                                    start=False, stop=(q == n_mm))
                    epilogue(t, ps)

            # ---------------- layer 1 agg -> z1hat = d_inv*relu(d_inv*agg+b1)
            def epi1(t, ps):
                t1 = zpool.tile([128, CHID], _F32, tag="t1")
                nc.vector.tensor_scalar(
                    out=t1[:], in0=ps[:], scalar1=dinv[:, t:t + 1],
                    scalar2=None, op0=mybir.AluOpType.mult)
                if not skip_bias:
                    nc.vector.tensor_tensor(
                        out=t1[:], in0=t1[:], in1=b1_sb[:],
                        op=mybir.AluOpType.add)
                # relu(dinv * t1) == dinv * relu(t1) since dinv > 0
                nc.scalar.activation(
                    zh_all[:, t * 128:(t + 1) * 128], t1[:],
                    mybir.ActivationFunctionType.Relu,
                    scale=dinv[:, t:t + 1])
                s = _chunk_of_tile(t)
                r0 = (t - TSTART[s]) * 128
                nc.sync.dma_start(agin2[s][r0:r0 + 128, :],
                                  zh_all[:, t * 128:(t + 1) * 128])

            PREF = 3

            def run_layer(tabs, loc_all, epilogue):
                # s-major prefetch: early pairs' gathers fire as each AG
                # chunk lands instead of serializing behind AG chunk 3
                pre = {pp: [None] * NSUB for pp in range(PREF)}
                for ss in range(NSUB):
                    for pp in range(PREF):
                        pre[pp][ss] = emit_gather(pp, ss, tabs)
                for pp in range(NPAIR):
                    gbufs = (pre[pp] if pp < PREF
                             else [emit_gather(pp, ss, tabs)
                                   for ss in range(NSUB)])
                    agg_pair(pp, gbufs, loc_all, epilogue)

            run_layer(h1tab, hh_all, epi1)

            for s in range(NSUB):
                nc.gpsimd.collective_compute(
                    "AllGather", mybir.AluOpType.bypass, replica_groups=rg,
                    ins=[agin2[s].opt()], outs=[h2tab[s].opt()])

            # ---------------- layer 2 agg (128-feat space) -> @W2 -> output
            def epi2(t, ps):
                aggS = zpool.tile([128, CHID], DT, tag="aggS")
                nc.vector.tensor_copy(aggS[:], ps[:])
                pst = ppool.tile([128, 128], DT, tag="ptr", bufs=2)
                nc.tensor.transpose(out=pst[:], in_=aggS[:],
                                    identity=ident1[:, :])
                aggT = hpool.tile([128, 128], DT, tag="aggT")
                nc.vector.tensor_copy(aggT[:], pst[:])
                ps2 = ppool.tile([128, COUT], _F32, tag="p2", bufs=2)
                nc.tensor.matmul(out=ps2[:], lhsT=aggT[:], rhs=w2b[:],
                                 start=True, stop=True)
                t2s = zpool.tile([128, COUT], _F32, tag="t2s")
                nc.vector.tensor_scalar(
                    out=t2s[:], in0=ps2[:], scalar1=dinv[:, t:t + 1],
                    scalar2=None, op0=mybir.AluOpType.mult)
                if not skip_bias:
                    nc.vector.tensor_tensor(
                        out=t2s[:], in0=t2s[:], in1=b2_sb[:],
                        op=mybir.AluOpType.add)
                z2 = zpool.tile([128, COUT], _F32, tag="z2")
                nc.scalar.activation(
                    z2[:], t2s[:], mybir.ActivationFunctionType.Relu)
                nc.sync.dma_start(z_t[t * 128:(t + 1) * 128, :], z2[:])

            run_layer(h2tab, zh_all, epi2)

    nc.compile()
    return nc


# ---------------------------------------------------------------- entry
_last_results = None


def kernel(x, edge_index, W1, b1, W2, b2):
    global _last_results
    x = np.asarray(x, np.float32)
    edge_index = np.asarray(edge_index)
    W1 = np.asarray(W1, np.float32)
    b1 = np.asarray(b1, np.float32)
    W2 = np.asarray(W2, np.float32)
    b2 = np.asarray(b2, np.float32)

    (idx_arrs, dstl_arrs, deg_cols, gather_sched, tile_sched,
     n_dstl, total_slots, kmax_g, kmax_sel) = _prep(edge_index)
    skip_bias = not (np.any(b1) or np.any(b2))
    nc = _build(gather_sched, tile_sched, n_dstl, total_slots, kmax_g,
                kmax_sel, skip_bias)

    xt = np.zeros((128, NP), np.float32)
    xt[:, :N] = x.T
    xt = xt.astype(ml_dtypes.bfloat16)
    b1_tile = np.ascontiguousarray(np.tile(b1.reshape(1, -1), (128, 1)))
    b2_tile = np.ascontiguousarray(np.tile(b2.reshape(1, -1), (128, 1)))
    ident_host = np.eye(128, dtype=np.float32)
    iotar_host = np.ascontiguousarray(
        np.tile(np.arange(128, dtype=np.float32), (128, kmax_sel)))
    in_maps = []
    for c in range(NCORES):
        in_maps.append({
            "xt": np.ascontiguousarray(xt[:, c * B:(c + 1) * B]),
            "idx": idx_arrs[c],
            "dstl": dstl_arrs[c],
            "deg": deg_cols[c],
            "w1": W1,
            "w2": W2,
            "b1": b1_tile,
            "b2": b2_tile,
            "ident": ident_host,
            "iotar": iotar_host,
        })

    trace = bool(os.environ.get("BASS_TRACE"))
    res = bass_utils.run_bass_kernel_spmd(
        nc, in_maps, core_ids=list(range(NCORES)), trace=trace)
    _last_results = res

    z = np.concatenate([res.results[c]["z"] for c in range(NCORES)], axis=0)
    return np.ascontiguousarray(z[:N], dtype=np.float32)


# revision 26
# speedup vs baseline: 1.0168x; 1.0168x over previous
"""Two-layer GCN (PyG GCNConv semantics) on 8 Trainium2 NeuronCores.

Sharding: nodes are partitioned into 8 contiguous blocks of B=12544 (padded
N=100352); core c owns node block c and all edges whose dst is in the block.
Both layers aggregate in 128-feature space (layer 2 uses linearity:
sum_e norm*(z1@W2)[src] == (sum_e norm*z1[src])@W2), so every gather table,
AllGather and selection matmul is bf16 with 256B rows:

    hhat1 = d_inv[:,None] * (x @ W1)
    z1hat = d_inv * relu(d_inv * (sum hhat1[src] + hhat1[v]) + b1)
    aggh2 = sum z1hat[src] + z1hat[v]
    z     = relu(d_inv * (aggh2 @ W2) + b2)

Edges are packed tight per (dst-tile-pair, src-sub) group and padded with
idx=-1: the SWDGE gather ucode trims trailing negative indices at runtime,
so per-core padding costs no descriptors and no DMA. Selection one-hots are
built with ONE DVE is_equal per dst tile: the dstl table stores
dst-slot-in-pair minus tile_rel*128 per (tile, sub, chunk) column, so each
tile compares against the same 0..127 iota.
"""

import os
import sys

sys.path.insert(0, "/opt/trn_rl_repo")

import ml_dtypes
import numpy as np

import concourse.bacc as bacc
import concourse.tile as tile
from concourse import bass_utils, mybir
from concourse.library_config import mlp

# ---------------------------------------------------------------------------
# Tile assigns Pool-engine (SWDGE) DMAs to the 8 DMASW semaphore lanes
# round-robin, ignoring queue_num. DMAs on different SWDGE queues complete
# out of order relative to each other, so a lane shared by two queues makes
# the cumulative wait thresholds unsound (CoreSim flags exactly this).
# Patch the lane assignment so each queue owns two dedicated lanes.
import concourse.tile_sem_assignment as _tsa
from concourse.tile_scheduler import DMAInst as _DMAInst

if not getattr(_tsa.TileClockTick, "_qaware_patched", False):
    _orig_assign_tick = _tsa.TileClockTick._assign_tick

    def _assign_tick_qaware(self, inst):
        from concourse import bass_isa as _bisa, mybir as _mb
        if (
            isinstance(inst, _DMAInst)
            and not isinstance(inst, _bisa.UserSyncedRemoteDMADescs)
            and inst.engine == _mb.EngineType.Pool
            and self.swdge_sem_count == 8
        ):
            q = int(getattr(inst, "queue_num", 0) or 0) % 4
            cnts = getattr(self, "_q_lane_cnt", None)
            if cnts is None:
                cnts = self._q_lane_cnt = [0, 0, 0, 0]
            self.next_sw_dma_idx = q * 2 + (cnts[q] % 2)
            cnts[q] += 1
        return _orig_assign_tick(self, inst)

    _tsa.TileClockTick._assign_tick = _assign_tick_qaware
    _tsa.TileClockTick._qaware_patched = True
# ---------------------------------------------------------------------------

# ---------------------------------------------------------------- constants
N = 100000
CIN, CHID, COUT = 128, 128, 64
NCORES = 8
B = 12544                 # nodes per core (98 tiles of 128)
NP = NCORES * B           # padded node count = 100352
TILES = B // 128          # 98 dst tiles per core
TPC = [25, 25, 24, 24]    # tiles per AllGather chunk (sum = 98)
TSTART = [0, 25, 50, 74]  # chunk start tile
NSUB = 4                  # gather sub-tables == AG chunks
PAIR = 4                  # dst tiles merged per gather instruction
NPAIR = -(-TILES // PAIR)
DT = mybir.dt.bfloat16

_F32 = mybir.dt.float32
_I16 = mybir.dt.int16
_CUM_TPC = np.cumsum(TPC)
PAD_DSTL = 999.0


def _chunk_of_tile(t):
    return int(np.searchsorted(_CUM_TPC, t, side="right"))


# ---------------------------------------------------------------- host prep
def _prep(edge_index):
    """Pack edges tight per (core, pair, sub); build per-core idx/dstl and a
    static (shared) gather + matmul schedule."""
    src = edge_index[0].astype(np.int64)
    dst = edge_index[1].astype(np.int64)
    E = src.shape[0]

    deg = np.bincount(dst, minlength=NP).astype(np.float32) + 1.0

    core = dst // B
    dstl = dst - core * B
    t = dstl >> 7
    slot = dstl & 127
    p = t // PAIR
    t_rel = t - p * PAIR
    dip = t_rel * 128 + slot                    # dst-slot-in-pair [0,512)

    csrc = src // B
    lsrc = src - csrc * B
    tsrc = lsrc >> 7
    psrc = lsrc & 127
    s = np.searchsorted(_CUM_TPC, tsrc, side="right")
    tpc_arr = np.asarray(TPC)
    tstart_arr = np.asarray(TSTART)
    row = csrc * (tpc_arr[s] * 128) + (tsrc - tstart_arr[s]) * 128 + psrc

    # group (core, pair, sub); edges sorted by group then tile (stable)
    gid = (core * NPAIR + p) * NSUB + s
    order = np.lexsort((t, gid))
    gid_s = gid[order]
    t_s = t[order]
    row_s = row[order]
    dip_s = dip[order]
    s_s = s[order]

    ngroups = NCORES * NPAIR * NSUB
    cnt = np.bincount(gid_s, minlength=ngroups).reshape(NCORES, NPAIR, NSUB)
    K = np.maximum(1, -(-cnt.max(axis=0) // 128))            # [NPAIR, NSUB]
    kmax_g = int(K.max())

    slot_base = np.zeros((NPAIR, NSUB), np.int64)
    acc = 0
    for pp in range(NPAIR):
        for ss in range(NSUB):
            slot_base[pp, ss] = acc
            acc += K[pp, ss] * 128
    total_slots = acc

    grp_start = np.zeros(ngroups + 1, np.int64)
    np.cumsum(cnt.reshape(-1), out=grp_start[1:])
    pos = np.arange(E) - grp_start[gid_s]
    # flat slot per (sorted) edge
    pp_e = (gid_s // NSUB) % NPAIR
    ss_e = gid_s % NSUB
    flat = slot_base[pp_e, ss_e] + pos
    core_e = gid_s // (NPAIR * NSUB)

    # per-core per (t, s): start/end position inside the (p, s) group
    cnt_ts = np.zeros((NCORES, TILES, NSUB), np.int64)
    np.add.at(cnt_ts, (core_e, t_s, s_s), 1)
    start_ts = np.zeros_like(cnt_ts)
    for pp in range(NPAIR):
        tt0 = pp * PAIR
        tt1 = min(tt0 + PAIR, TILES)
        run = np.zeros((NCORES, NSUB), np.int64)
        for tt in range(tt0, tt1):
            start_ts[:, tt, :] = run
            run += cnt_ts[:, tt, :]
    end_ts = start_ts + cnt_ts

    # static union chunk range per (t, s) over cores with edges
    any_c = cnt_ts > 0                                        # [C, T, S]
    lo_all = start_ts // 128
    hi_all = -(-end_ts // 128)
    lo_ts = np.where(any_c, lo_all, 10 ** 9).min(axis=0)      # [T, S]
    hi_ts = np.where(any_c, hi_all, -1).max(axis=0)
    empty = ~any_c.any(axis=0)
    lo_ts[empty] = 0
    hi_ts[empty] = 0
    rng_ts = (hi_ts - lo_ts).astype(np.int64)                 # [T, S]
    n_mm_t = rng_ts.sum(axis=1)                               # [T]
    kmax_sel = int(n_mm_t.max())
    selbase_t = np.zeros(TILES + 1, np.int64)
    np.cumsum(n_mm_t, out=selbase_t[1:])
    n_dstl = int(selbase_t[-1])

    # dstl column -> (global chunk index, value offset) map (static)
    chunk_map = np.zeros(n_dstl, np.int64)
    off_map = np.zeros(n_dstl, np.float32)
    for tt in range(TILES):
        pp = tt // PAIR
        c0 = selbase_t[tt]
        for ss in range(NSUB):
            r = rng_ts[tt, ss]
            if r == 0:
                continue
            gchunk0 = slot_base[pp, ss] // 128 + lo_ts[tt, ss]
            chunk_map[c0:c0 + r] = np.arange(gchunk0, gchunk0 + r)
            off_map[c0:c0 + r] = (tt % PAIR) * 128
            c0 += r

    # per-core arrays
    idx_arrs, dstl_arrs, deg_cols = [], [], []
    w16 = total_slots // 16
    n_chunks_tot = total_slots // 128
    for c in range(NCORES):
        mask = core_e == c
        fl = flat[mask]
        idx_flat = np.full(total_slots, 0, np.int16)
        dip_full = np.full(total_slots, PAD_DSTL, np.float32)
        idx_flat[fl] = row_s[mask].astype(np.int16)
        dip_full[fl] = dip_s[mask].astype(np.float32)

        iw = idx_flat.reshape(w16, 16).T                      # [16, w16]
        idx_arrs.append(np.tile(iw, (8, 1)).astype(np.int16))

        chunks = dip_full.reshape(n_chunks_tot, 128)          # [chunks, 128]
        dcols = chunks[chunk_map].T - off_map[None, :]        # [128, n_dstl]
        dstl_arrs.append(np.ascontiguousarray(dcols.astype(np.float32)))

        deg_cols.append(np.ascontiguousarray(
            deg[c * B:(c + 1) * B].reshape(TILES, 128).T))

    gather_sched = [[(int(slot_base[pp, ss]), int(K[pp, ss]))
                     for ss in range(NSUB)] for pp in range(NPAIR)]
    tile_sched = [
        dict(selbase=int(selbase_t[tt]), n_mm=int(n_mm_t[tt]),
             rng=[(int(lo_ts[tt, ss]), int(rng_ts[tt, ss]))
                  for ss in range(NSUB)])
        for tt in range(TILES)
    ]
    return (idx_arrs, dstl_arrs, deg_cols, gather_sched, tile_sched,
            n_dstl, total_slots, kmax_g, kmax_sel)


# ---------------------------------------------------------------- device IR
def _build(gather_sched, tile_sched, n_dstl, total_slots, kmax_g, kmax_sel,
           skip_bias):
    nc = bacc.Bacc(
        "TRN2",
        target_bir_lowering=False,
        debug=False,
        num_devices=NCORES,
        num_swdge_queues=4,
    )

    w16 = total_slots // 16
    xt_t = nc.dram_tensor("xt", [128, B], DT, kind="ExternalInput")
    idx_t = nc.dram_tensor("idx", [128, w16], _I16, kind="ExternalInput")
    dstl_t = nc.dram_tensor("dstl", [128, n_dstl], _F32, kind="ExternalInput")
    deg_t = nc.dram_tensor("deg", [128, TILES], _F32, kind="ExternalInput")
    w1_t = nc.dram_tensor("w1", [CIN, CHID], _F32, kind="ExternalInput")
    w2_t = nc.dram_tensor("w2", [CHID, COUT], _F32, kind="ExternalInput")
    b1_t = nc.dram_tensor("b1", [128, CHID], _F32, kind="ExternalInput")
    b2_t = nc.dram_tensor("b2", [128, COUT], _F32, kind="ExternalInput")
    ident_t = nc.dram_tensor("ident", [128, 128], _F32, kind="ExternalInput")
    iotar_t = nc.dram_tensor("iotar", [128, kmax_sel * 128], _F32,
                             kind="ExternalInput")
    z_t = nc.dram_tensor("z", [B, COUT], _F32, kind="ExternalOutput")

    rg = [list(range(NCORES))]

    with tile.TileContext(nc) as tc:
        with (
            tc.tile_pool(name="const", bufs=1) as cpool,
            tc.tile_pool(name="sel", bufs=3) as spool,
            tc.tile_pool(name="gath", bufs=4) as gpool,
            tc.tile_pool(name="zeps", bufs=4) as zpool,
            tc.tile_pool(name="hh2", bufs=3) as hpool,
            tc.tile_pool(name="ps", bufs=2, space="PSUM") as ppool,
            tc.tile_pool(name="dram", bufs=1, space="DRAM") as dpool,
        ):
            nc.gpsimd.load_library(mlp)

            # ---- constants / inputs staged once
            idx_sb = cpool.tile([128, w16], _I16)
            nc.sync.dma_start(idx_sb[:], idx_t[:])
            dstl_f = cpool.tile([128, n_dstl], _F32)
            nc.sync.dma_start(dstl_f[:], dstl_t[:])
            dstl_sb = cpool.tile([128, n_dstl], DT)
            nc.vector.tensor_copy(dstl_sb[:], dstl_f[:])
            deg_sb = cpool.tile([128, TILES], _F32)
            nc.sync.dma_start(deg_sb[:], deg_t[:])
            w1_sb = cpool.tile([CIN, CHID], _F32)
            nc.sync.dma_start(w1_sb[:], w1_t[:])
            w2_sb = cpool.tile([CHID, COUT], _F32)
            nc.sync.dma_start(w2_sb[:], w2_t[:])
            w1b = cpool.tile([CIN, CHID], DT)
            nc.vector.tensor_copy(w1b[:], w1_sb[:])
            w2b = cpool.tile([CHID, COUT], DT)
            nc.vector.tensor_copy(w2b[:], w2_sb[:])
            b1_sb = cpool.tile([128, CHID], _F32)
            nc.sync.dma_start(b1_sb[:], b1_t[:])
            b2_sb = cpool.tile([128, COUT], _F32)
            nc.sync.dma_start(b2_sb[:], b2_t[:])
            identF = cpool.tile([128, 128], _F32)
            nc.sync.dma_start(identF[:], ident_t[:])
            ident1 = cpool.tile([128, 128], DT)
            nc.vector.tensor_copy(ident1[:], identF[:])
            iotaF = cpool.tile([128, kmax_sel * 128], _F32)
            nc.sync.dma_start(iotaF[:], iotar_t[:])
            iota1 = cpool.tile([128, kmax_sel * 128], DT)
            nc.vector.tensor_copy(iota1[:], iotaF[:])

            # d_inv = sqrt(1/deg)
            recip = cpool.tile([128, TILES], _F32)
            nc.vector.reciprocal(recip[:], deg_sb[:])
            dinv = cpool.tile([128, TILES], _F32)
            nc.scalar.activation(dinv[:], recip[:],
                                 mybir.ActivationFunctionType.Sqrt)

            # SBUF-resident x^T, hhat1, z1hat (per-tile slices)
            xfull = cpool.tile([128, B], DT)
            for ss in range(NSUB):
                a = TSTART[ss] * 128
                bb = a + TPC[ss] * 128
                nc.sync.dma_start(xfull[:, a:bb], xt_t[:, a:bb])
            hh_all = cpool.tile([128, B], DT)
            zh_all = cpool.tile([128, B], DT)

            # warm the gather buffers so runtime-trimmed (unwritten) slots
            # hold finite values (sel==0 against NaN garbage would poison
            # PSUM)
            gwarm = []
            for ss in range(NSUB):
                for _ in range(4):
                    g_sb = gpool.tile([128, kmax_g, CHID], DT, tag=f"g{ss}")
                    nc.vector.memset(g_sb[:], 0)
                    gwarm.append(g_sb)

            # ---- DRAM buffers: AG inputs (local tables) and gather tables
            agin1 = [dpool.tile([TPC[s] * 128, CHID], DT, name=f"agin1_{s}")
                     for s in range(NSUB)]
            h1tab = [dpool.tile([NCORES * TPC[s] * 128, CHID], DT,
                                name=f"h1tab_{s}") for s in range(NSUB)]
            agin2 = [dpool.tile([TPC[s] * 128, CHID], DT, name=f"agin2_{s}")
                     for s in range(NSUB)]
            h2tab = [dpool.tile([NCORES * TPC[s] * 128, CHID], DT,
                                name=f"h2tab_{s}") for s in range(NSUB)]

            # ---------------- phase 1: hhat1 = d_inv * (x @ W1), local rows
            for t in range(TILES):
                ps = ppool.tile([128, CHID], _F32, tag="pagg", bufs=4)
                nc.tensor.matmul(out=ps[:],
                                 lhsT=xfull[:, t * 128:(t + 1) * 128],
                                 rhs=w1b[:], start=True, stop=True)
                nc.vector.tensor_scalar(
                    out=hh_all[:, t * 128:(t + 1) * 128], in0=ps[:],
                    scalar1=dinv[:, t:t + 1],
                    scalar2=None, op0=mybir.AluOpType.mult)
                s = _chunk_of_tile(t)
                r0 = (t - TSTART[s]) * 128
                nc.sync.dma_start(agin1[s][r0:r0 + 128, :],
                                  hh_all[:, t * 128:(t + 1) * 128])

            for s in range(NSUB):
                nc.gpsimd.collective_compute(
                    "AllGather", mybir.AluOpType.bypass, replica_groups=rg,
                    ins=[agin1[s].opt()], outs=[h1tab[s].opt()])

            # ---------------- aggregation layer (per pair of dst tiles)
            def emit_gather(pp, ss, tabs):
                sbase, kps = gather_sched[pp][ss]
                g_sb = gpool.tile([128, kmax_g, CHID], DT, tag=f"g{ss}")
                o16 = sbase // 16
                nc.gpsimd.dma_gather(
                    g_sb[:, :kps, :], tabs[ss][:],
                    idx_sb[:, o16:o16 + kps * 8],
                    kps * 128, kps * 128, CHID,
                    single_packet=False, queue_num=(pp + ss) % 4)
                return g_sb

            def agg_pair(pp, gbufs, loc_all, epilogue):
                for t in range(pp * PAIR, min((pp + 1) * PAIR, TILES)):
                    ts = tile_sched[t]
                    n_mm = ts["n_mm"]
                    ps = ppool.tile([128, CHID], _F32, tag="pagg", bufs=4)
                    # self-loop (outer d_inv scale supplies the square)
                    nc.tensor.matmul(out=ps[:], lhsT=ident1[:],
                                     rhs=loc_all[:, t * 128:(t + 1) * 128],
                                     start=True, stop=(n_mm == 0))
                    if n_mm:
                        sb = ts["selbase"]
                        sel = spool.tile([128, kmax_sel, 128], DT, tag="sel")
                        nc.vector.tensor_tensor(
                            out=sel[:, :n_mm, :],
                            in0=iota1[:, :n_mm * 128].rearrange(
                                "p (k c) -> p k c", c=128),
                            in1=dstl_sb[:, sb:sb + n_mm].to_broadcast(
                                [128, n_mm, 128]),
                            op=mybir.AluOpType.is_equal)
                        q = 0
                        for ss in range(NSUB):
                            lo, r = ts["rng"][ss]
                            for jj in range(r):
                                q += 1
                                nc.tensor.matmul(
                                    out=ps[:], lhsT=sel[:, q - 1, :],
                                    rhs=gbufs[ss][:, lo + jj, :],
                                    start=False, stop=(q == n_mm))
                    epilogue(t, ps)

            # ---------------- layer 1 agg -> z1hat = d_inv*relu(d_inv*agg+b1)
            def epi1(t, ps):
                t1 = zpool.tile([128, CHID], _F32, tag="t1")
                nc.vector.tensor_scalar(
                    out=t1[:], in0=ps[:], scalar1=dinv[:, t:t + 1],
                    scalar2=None, op0=mybir.AluOpType.mult)
                if not skip_bias:
                    nc.vector.tensor_tensor(
                        out=t1[:], in0=t1[:], in1=b1_sb[:],
                        op=mybir.AluOpType.add)
                # relu(dinv * t1) == dinv * relu(t1) since dinv > 0
                nc.scalar.activation(
                    zh_all[:, t * 128:(t + 1) * 128], t1[:],
                    mybir.ActivationFunctionType.Relu,
                    scale=dinv[:, t:t + 1])
                s = _chunk_of_tile(t)
                r0 = (t - TSTART[s]) * 128
                nc.sync.dma_start(agin2[s][r0:r0 + 128, :],
                                  zh_all[:, t * 128:(t + 1) * 128])

            PREF = 4

            def run_layer(tabs, loc_all, epilogue):
                # s-major prefetch: early pairs' gathers fire as each AG
                # chunk lands instead of serializing behind AG chunk 3
                pre = {pp: [None] * NSUB for pp in range(PREF)}
                for ss in range(NSUB):
                    for pp in range(PREF):
                        pre[pp][ss] = emit_gather(pp, ss, tabs)
                for pp in range(NPAIR):
                    gbufs = (pre[pp] if pp < PREF
                             else [emit_gather(pp, ss, tabs)
                                   for ss in range(NSUB)])
                    agg_pair(pp, gbufs, loc_all, epilogue)

            run_layer(h1tab, hh_all, epi1)

            for s in range(NSUB):
                nc.gpsimd.collective_compute(
                    "AllGather", mybir.AluOpType.bypass, replica_groups=rg,
                    ins=[agin2[s].opt()], outs=[h2tab[s].opt()])

            # ---------------- layer 2 agg (128-feat space) -> @W2 -> output
            def epi2(t, ps):
                aggS = zpool.tile([128, CHID], DT, tag="aggS")
                nc.vector.tensor_copy(aggS[:], ps[:])
                pst = ppool.tile([128, 128], DT, tag="ptr", bufs=2)
                nc.tensor.transpose(out=pst[:], in_=aggS[:],
                                    identity=ident1[:, :])
                aggT = hpool.tile([128, 128], DT, tag="aggT")
                nc.vector.tensor_copy(aggT[:], pst[:])
                ps2 = ppool.tile([128, COUT], _F32, tag="p2", bufs=2)
                nc.tensor.matmul(out=ps2[:], lhsT=aggT[:], rhs=w2b[:],
                                 start=True, stop=True)
                t2s = zpool.tile([128, COUT], _F32, tag="t2s")
                nc.vector.tensor_scalar(
                    out=t2s[:], in0=ps2[:], scalar1=dinv[:, t:t + 1],
                    scalar2=None, op0=mybir.AluOpType.mult)
                if not skip_bias:
                    nc.vector.tensor_tensor(
                        out=t2s[:], in0=t2s[:], in1=b2_sb[:],
                        op=mybir.AluOpType.add)
                z2 = zpool.tile([128, COUT], _F32, tag="z2")
                nc.scalar.activation(
                    z2[:], t2s[:], mybir.ActivationFunctionType.Relu)
                nc.sync.dma_start(z_t[t * 128:(t + 1) * 128, :], z2[:])

            run_layer(h2tab, zh_all, epi2)

    nc.compile()
    return nc


# ---------------------------------------------------------------- entry
_last_results = None


def kernel(x, edge_index, W1, b1, W2, b2):
    global _last_results
    x = np.asarray(x, np.float32)
    edge_index = np.asarray(edge_index)
    W1 = np.asarray(W1, np.float32)
    b1 = np.asarray(b1, np.float32)
    W2 = np.asarray(W2, np.float32)
    b2 = np.asarray(b2, np.float32)

    (idx_arrs, dstl_arrs, deg_cols, gather_sched, tile_sched,
     n_dstl, total_slots, kmax_g, kmax_sel) = _prep(edge_index)
    skip_bias = not (np.any(b1) or np.any(b2))
    nc = _build(gather_sched, tile_sched, n_dstl, total_slots, kmax_g,
                kmax_sel, skip_bias)

    xt = np.zeros((128, NP), np.float32)
    xt[:, :N] = x.T
    xt = xt.astype(ml_dtypes.bfloat16)
    b1_tile = np.ascontiguousarray(np.tile(b1.reshape(1, -1), (128, 1)))
    b2_tile = np.ascontiguousarray(np.tile(b2.reshape(1, -1), (128, 1)))
    ident_host = np.eye(128, dtype=np.float32)
    iotar_host = np.ascontiguousarray(
        np.tile(np.arange(128, dtype=np.float32), (128, kmax_sel)))
    in_maps = []
    for c in range(NCORES):
        in_maps.append({
            "xt": np.ascontiguousarray(xt[:, c * B:(c + 1) * B]),
            "idx": idx_arrs[c],
            "dstl": dstl_arrs[c],
            "deg": deg_cols[c],
            "w1": W1,
            "w2": W2,
            "b1": b1_tile,
            "b2": b2_tile,
            "ident": ident_host,
            "iotar": iotar_host,
        })

    trace = bool(os.environ.get("BASS_TRACE"))
    res = bass_utils.run_bass_kernel_spmd(
        nc, in_maps, core_ids=list(range(NCORES)), trace=trace)
    _last_results = res

    z = np.concatenate([res.results[c]["z"] for c in range(NCORES)], axis=0)
    return np.ascontiguousarray(z[:N], dtype=np.float32)
